# revision 1
# baseline (speedup 1.0000x reference)
"""Trainium2 Bass kernel for nn_ConnectLoss (ConnectLoss: BCE-on-connectivity +
edge min-prob loss + bilateral-voting dice loss).

Strategy: pure data parallel — one batch element per NeuronCore (B=8 on 8
cores). The one-pixel-shift translation matmuls of the reference are realized
as zero-padded shifted copies (DMA), turning the whole problem into
elementwise work + global reductions:

  conn_loss = [ -sum log(1-sig) - sum_d <t*shift_d(t), x_d> ] / N
  edge_loss = -sum log(1-pm) / sum pm,   pm = min_d sig_d * edge(t)
  seg_loss  = mean_b (1 - (2*I_b+1)/(U_b+1)),  via final = max_d sig_d*shift(sig_{7-d})

Per-core device program reduces everything to a [128, NSTAT] stats tile
(per-partition partial sums via DVE/ACT accumulators); host combines in f64.

Self-contained: only needs numpy + the in-container concourse stack.
"""
import numpy as np
from contextlib import ExitStack

B, CHN, H, W = 8, 8, 512, 512
NCORES = 8
P = 128
NCH = H // P          # 4 row chunks of 128 partitions
WP = W + 4            # padded chunk width (zeros at cols 1 and 514 are read)
OFF = 2               # center column offset (4-byte aligned for bf16)
# direction d -> (dr, dc): shifted[h, w] = src[h+dr, w+dc], zero outside
DIRS = [(-1, -1), (-1, 0), (-1, 1), (0, -1), (0, 1), (1, -1), (1, 0), (1, 1)]

# stats columns
NSTAT = 24
S_CROSS = 0    # sum_d sum_px conn_d * x_d  (PE ones-matmul, partition 0 only)
S_LOG = 8      # 8..15  sum_px log(1 - sig_d)          (ACT Ln accum)
S_LOGPM = 16   # sum_px log(1 - pm)
S_DEN = 17     # sum_px pm
S_T = 18       # sum_px t
S_FIN = 19     # sum_px final
S_FINT = 20    # sum_px final * t

_CACHE: dict = {}


def _emit(tc, pred_ap, tgt_ap, stats_ap, level=99):
    import concourse.bass as bass  # noqa: F401
    from concourse import mybir
    from concourse.tile_rust import add_dep_helper

    nc = tc.nc
    f32, bf16 = mybir.dt.float32, mybir.dt.bfloat16
    Alu = mybir.AluOpType
    Act = mybir.ActivationFunctionType

    with ExitStack() as ctx:
        pers = ctx.enter_context(tc.tile_pool(name="pers", bufs=1))
        xpool = ctx.enter_context(tc.tile_pool(name="x", bufs=3))
        sp = ctx.enter_context(tc.tile_pool(name="sp", bufs=1))

        sig = [pers.tile([P, NCH, WP], bf16, name=f"sig{d}", tag=f"sig{d}") for d in range(8)]
        sv = [pers.tile([P, NCH, W], bf16, name=f"sv{d}", tag=f"sv{d}") for d in range(8)]
        st = [pers.tile([P, NCH, W], bf16, name=f"st{d}", tag=f"st{d}") for d in range(8)]
        t0 = pers.tile([P, NCH, WP], bf16, name="t0", tag="t0")
        stats = pers.tile([P, NSTAT], f32, name="statsT", tag="statsT")

        def ctr(tl):  # center view of a padded plane
            return tl[:, :, OFF:OFF + W]

        nc.vector.memset(stats[:], 0.0)
        # zero the pad columns that shifted reads touch (cols 1 and 514)
        for tl in [t0] + sig:
            nc.vector.memset(tl[:, :, OFF - 1:OFF], 0.0)
            nc.vector.memset(tl[:, :, OFF + W:OFF + W + 1], 0.0)

        zrow = pers.tile([1, W], bf16, name="zrow", tag="zrow")
        nc.vector.memset(zrow[:], 0.0)

        def shift_copy(dst, src, dr, dc):
            """dst[h, w] = src_plane[h+dr, w+dc]; col boundary zeros come from
            src pad columns, row boundary zeroed explicitly."""
            s0 = OFF + dc
            if dr == 0:
                nc.sync.dma_start(out=dst[:, :, 0:W], in_=src[:, :, s0:s0 + W])
            elif dr == 1:
                nc.sync.dma_start(out=dst[0:P - 1, :, 0:W],
                                  in_=src[1:P, :, s0:s0 + W])
                nc.sync.dma_start(out=dst[P - 1:P, 0:NCH - 1, 0:W],
                                  in_=src[0:1, 1:NCH, s0:s0 + W])
                nc.sync.dma_start(out=dst[P - 1:P, NCH - 1, 0:W], in_=zrow[:])
            else:
                nc.sync.dma_start(out=dst[1:P, :, 0:W],
                                  in_=src[0:P - 1, :, s0:s0 + W])
                nc.sync.dma_start(out=dst[0:1, 1:NCH, 0:W],
                                  in_=src[P - 1:P, 0:NCH - 1, s0:s0 + W])
                nc.vector.memset(dst[0:1, 0:1, 0:W], 0.0)

        # ---- target: load, cast to bf16, shifted copies -------------------
        tgt_f = sp.tile([P, NCH, W], f32, name="tgt", tag="tgt")
        nc.sync.dma_start(out=tgt_f[:], in_=tgt_ap.rearrange("(c p) w -> p c w", p=P))
        nc.vector.tensor_copy(ctr(t0), tgt_f[:])
        t0c = ctr(t0)
        if level >= 2:
            for d, (dr, dc) in enumerate(DIRS):
                shift_copy(st[d], t0, dr, dc)

        # ---- pred: load, sigmoid, BCE cross term --------------------------
        # cross = sum_d <t * shift_d(t), x_d>, accumulated on the (otherwise
        # idle) TensorEngine as ones^T @ w into one PSUM row.
        ones = pers.tile([P, 1], bf16, name="ones", tag="ones")
        nc.vector.memset(ones[:], 1.0)
        psum_pool = ctx.enter_context(
            tc.tile_pool(name="ps", bufs=1, space="PSUM"))
        ps_cross = psum_pool.tile([1, W], f32, name="ps_cross", tag="ps_cross")
        conn = sp.tile([P, NCH, W], bf16, name="conn", tag="conn")
        wmul = sp.tile([P, NCH, W], bf16, name="wmul", tag="wmul")
        sig_insts = []
        mm_idx, mm_total = 0, 8 * NCH
        for d in range(8):
            x = xpool.tile([P, NCH, W], f32, name=f"x{d}", tag="x")
            nc.sync.dma_start(out=x[:], in_=pred_ap[d].rearrange("(c p) w -> p c w", p=P))
            sig_insts.append(
                nc.scalar.activation(ctr(sig[d]), x[:], Act.Sigmoid))
            if level >= 4:
                nc.vector.tensor_mul(conn[:], t0c, st[d][:])
                nc.vector.tensor_mul(wmul[:], conn[:], x[:])
                for c in range(NCH):
                    nc.tensor.matmul(ps_cross[:], ones[:], wmul[:, c, :],
                                     start=(mm_idx == 0),
                                     stop=(mm_idx == mm_total - 1))
                    mm_idx += 1

        if level >= 4:
            # collapse the PSUM cross row to a scalar in stats (partition 0)
            nc.vector.tensor_reduce(out=stats[0:1, S_CROSS:S_CROSS + 1],
                                    in_=ps_cross[:], axis=mybir.AxisListType.X,
                                    op=Alu.add)

        # ---- shifted sigmoid planes for voting ----------------------------
        if level >= 2:
            for d, (dr, dc) in enumerate(DIRS):
                shift_copy(sv[d], sig[7 - d], dr, dc)

        # ---- neighbor count + edge mask -----------------------------------
        if level < 3:
            nc.sync.dma_start(out=stats_ap, in_=stats[:])
            return
        cnt = sp.tile([P, NCH, W], bf16, name="cnt", tag="cnt")
        nc.vector.tensor_add(cnt[:], st[0][:], st[1][:])
        for d in range(2, 8):
            nc.vector.tensor_add(cnt[:], cnt[:], st[d][:])
        e1 = sp.tile([P, NCH, W], bf16, name="e1", tag="e1")
        e2 = sp.tile([P, NCH, W], bf16, name="e2", tag="e2")
        nc.vector.tensor_scalar(e1[:], cnt[:], 7.5, None, Alu.is_lt)
        nc.vector.tensor_scalar(e2[:], cnt[:], 0.5, None, Alu.is_gt)
        nc.vector.tensor_mul(e1[:], e1[:], e2[:])
        nc.vector.tensor_mul(e1[:], e1[:], t0c)      # e1 = edge * t

        # ---- pm = min_d sig * edge ----------------------------------------
        smin = sp.tile([P, NCH, W], bf16, name="smin", tag="smin")
        nc.vector.tensor_tensor(smin[:], ctr(sig[0]), ctr(sig[1]), Alu.min)
        for d in range(2, 8):
            nc.vector.tensor_tensor(smin[:], smin[:], ctr(sig[d]), Alu.min)
        pm = sp.tile([P, NCH, W], bf16, name="pm", tag="pm")
        nc.vector.tensor_mul(pm[:], smin[:], e1[:])

        tsout = sp.tile([P, NCH, W], bf16, name="tsout", tag="tsout")
        nc.vector.tensor_scalar(tsout[:], pm[:], 1.0, None, Alu.mult, Alu.add,
                                accum_out=stats[:, S_DEN:S_DEN + 1])
        nc.vector.tensor_scalar(tsout[:], t0c, 1.0, None, Alu.mult, Alu.add,
                                accum_out=stats[:, S_T:S_T + 1])

        # ---- bilateral voting ---------------------------------------------
        vacc = sp.tile([P, NCH, W], bf16, name="vacc", tag="vacc")
        vtmp = sp.tile([P, NCH, W], bf16, name="vtmp", tag="vtmp")
        nc.vector.tensor_mul(vacc[:], ctr(sig[0]), sv[0][:])
        for d in range(1, 8):
            nc.vector.tensor_mul(vtmp[:], ctr(sig[d]), sv[d][:])
            nc.vector.tensor_tensor(vacc[:], vacc[:], vtmp[:], Alu.max)
        nc.vector.tensor_scalar(tsout[:], vacc[:], 1.0, None, Alu.mult, Alu.add,
                                accum_out=stats[:, S_FIN:S_FIN + 1])
        nc.vector.tensor_mul(vtmp[:], vacc[:], t0c)
        nc.vector.tensor_scalar(tsout[:], vtmp[:], 1.0, None, Alu.mult, Alu.add,
                                accum_out=stats[:, S_FINT:S_FINT + 1])

        # ---- Ln phase (single act-table switch after all sigmoids) --------
        if level < 5:
            nc.sync.dma_start(out=stats_ap, in_=stats[:])
            return
        lout = sp.tile([P, NCH, W], bf16, name="lout", tag="lout")
        last_sig = sig_insts[-1]
        for d in range(8):
            ins = nc.scalar.activation(
                lout[:], ctr(sig[d]), Act.Ln, bias=1.0, scale=-1.0,
                accum_out=stats[:, S_LOG + d:S_LOG + d + 1])
            add_dep_helper(ins.ins, last_sig.ins, sync=False,
                           reason="batch act-table: Ln after all sigmoids")
        ins = nc.scalar.activation(
            lout[:], pm[:], Act.Ln, bias=1.0, scale=-1.0,
            accum_out=stats[:, S_LOGPM:S_LOGPM + 1])
        add_dep_helper(ins.ins, last_sig.ins, sync=False,
                       reason="batch act-table: Ln after all sigmoids")

        nc.sync.dma_start(out=stats_ap, in_=stats[:])


def _build_nc(repeat=1, level=99):
    import concourse.bacc as bacc
    import concourse.tile as tile
    from concourse import mybir

    nc = bacc.Bacc("TRN2", target_bir_lowering=False, debug=False,
                   enable_asserts=False, num_devices=NCORES)
    f32 = mybir.dt.float32
    pred_t = nc.dram_tensor("pred", [CHN, H, W], f32, kind="ExternalInput")
    tgt_t = nc.dram_tensor("target", [H, W], f32, kind="ExternalInput")
    stats_t = nc.dram_tensor("stats", [P, NSTAT], f32, kind="ExternalOutput")
    with tile.TileContext(nc) as tc:
        for _ in range(repeat):
            # the body is idempotent (accumulators overwrite), so repeated
            # emission supports overhead-cancelling wall-clock timing
            _emit(tc, pred_t.ap(), tgt_t.ap(), stats_t.ap(), level=level)
    nc.compile()
    return nc


def _get_nc():
    if "nc" not in _CACHE:
        _CACHE["nc"] = _build_nc()
    return _CACHE["nc"]


def _make_in_maps(pred, target):
    return [{"pred": np.ascontiguousarray(pred[b]),
             "target": np.ascontiguousarray(target[b, 0])} for b in range(B)]


def _combine(stats_list):
    s = np.stack([s.astype(np.float64) for s in stats_list])  # [B, P, NSTAT]
    cols = s.sum(axis=1)                                      # [B, NSTAT]
    cross = cols[:, S_CROSS].sum()
    slog = cols[:, S_LOG:S_LOG + 8].sum()
    slogpm = cols[:, S_LOGPM].sum()
    den = cols[:, S_DEN].sum()
    sum_t = cols[:, S_T]
    sum_fin = cols[:, S_FIN]
    sum_fint = cols[:, S_FINT]

    n_elem = B * CHN * H * W
    conn_loss = (-slog - cross) / n_elem
    edge_loss = -slogpm / den
    dice = (2.0 * sum_fint + 1.0) / (sum_fin + sum_t + 1.0)
    seg_loss = (1.0 - dice).mean()
    return np.asarray(conn_loss + edge_loss + seg_loss, dtype=np.float32)


def _is_shift_mats(hori, verti):
    hm = np.zeros((W, W), np.float32)
    hm[np.arange(W - 1), np.arange(1, W)] = 1.0
    vm = np.zeros((H, H), np.float32)
    vm[np.arange(H - 1), np.arange(1, H)] = 1.0
    return (np.array_equal(np.asarray(hori),
                           np.broadcast_to(hm, (B, 1, W, W))) and
            np.array_equal(np.asarray(verti),
                           np.broadcast_to(vm, (B, 1, H, H))))


def kernel(pred, target, hori_translation, verti_translation):
    pred = np.asarray(pred, dtype=np.float32)
    target = np.asarray(target, dtype=np.float32)
    if not _is_shift_mats(hori_translation, verti_translation):
        return _fallback(pred, target,
                         np.asarray(hori_translation, dtype=np.float32),
                         np.asarray(verti_translation, dtype=np.float32))

    from concourse.bass_utils import run_bass_kernel_spmd
    nc = _get_nc()
    res = run_bass_kernel_spmd(nc, _make_in_maps(pred, target),
                               list(range(NCORES)))
    return _combine([res.results[b]["stats"] for b in range(B)])


# ---------------------------------------------------------------------------
# Fallback for non-shift translation matrices: faithful numpy replica of the
# reference (never taken for the standard setup_inputs data).
def _fallback(pred, target, hori, verti):
    NEG_CLAMP = -100.0
    dt = np.float64
    predd, targetd = pred.astype(dt), target.astype(dt)
    horid, vertid = hori.astype(dt), verti.astype(dt)

    z = np.zeros_like(targetd)
    def sh(dr, dc):
        out = z.copy()
        hs = slice(max(0, -dr), H - max(0, dr))
        ws = slice(max(0, -dc), W - max(0, dc))
        hsrc = slice(max(0, dr), H + min(0, dr) if dr < 0 else H)
        wsrc = slice(max(0, dc), W + min(0, dc) if dc < 0 else W)
        out[..., hs, ws] = targetd[..., hsrc, wsrc]
        return out

    conn_t = np.stack([targetd * sh(dr, dc) for (dr, dc) in DIRS], axis=2)
    sigd = 1.0 / (1.0 + np.exp(-predd))
    with np.errstate(divide="ignore"):
        lp = np.maximum(np.log(sigd), NEG_CLAMP)
        l1p = np.maximum(np.log1p(-sigd), NEG_CLAMP)
    ct = conn_t.reshape(predd.shape)
    conn_loss = (-(ct * lp + (1.0 - ct) * l1p)).mean()

    sum_conn = conn_t.sum(axis=2)
    edge = ((sum_conn < 8) & (sum_conn > 0)).astype(dt)
    sig5 = sigd.reshape(B, 1, 8, H, W)
    pmin = np.min(sig5, axis=2) * edge
    edge_loss = (-np.maximum(np.log1p(-pmin), NEG_CLAMP)).sum() / pmin.sum()

    mm_h = lambda m, T: np.einsum('bchw,bcwv->bchv', m, T)
    mm_hT = lambda m, T: np.einsum('bchw,bcvw->bchv', m, T)
    mm_v = lambda T, m: np.einsum('bcrh,bchw->bcrw', T, m)
    mm_vT = lambda T, m: np.einsum('bchr,bchw->bcrw', T, m)
    c = sig5
    right = mm_h(c[:, :, 4], horid)
    left = mm_hT(c[:, :, 3], horid)
    bottom = mm_vT(vertid, c[:, :, 6])
    up = mm_v(vertid, c[:, :, 1])
    left_bottom = mm_hT(mm_vT(vertid, c[:, :, 5]), horid)
    right_above = mm_h(mm_v(vertid, c[:, :, 2]), horid)
    left_above = mm_hT(mm_v(vertid, c[:, :, 0]), horid)
    right_bottom = mm_h(mm_vT(vertid, c[:, :, 7]), horid)
    vote = np.stack([c[:, :, 0] * right_bottom, c[:, :, 1] * bottom,
                     c[:, :, 2] * left_bottom, c[:, :, 3] * right,
                     c[:, :, 4] * left, c[:, :, 5] * right_above,
                     c[:, :, 6] * up, c[:, :, 7] * left_above], axis=2)
    final_pred = vote.max(axis=2)
    inter = (final_pred * targetd).sum(axis=(2, 3))
    union = final_pred.sum(axis=(2, 3)) + targetd.sum(axis=(2, 3))
    dice = (2.0 * inter + 1.0) / (union + 1.0)
    seg_loss = (1.0 - dice).mean()
    return np.asarray(conn_loss + edge_loss + seg_loss, dtype=np.float32)

